# revision 17
# baseline (speedup 1.0000x reference)
"""MoE (noisy top-2 routing, 8 experts) on 8 Trainium2 NeuronCores.

Strategy (expert-parallel, per the sharding hint):
  - Router runs on host in float64 (134 MFLOP — negligible next to the
    137 GFLOP expert MLPs; fp64 makes the top-k selection robust).
  - Tokens are dispatched by top-2 expert id. v4 balances the per-core
    column count below the max expert count with an SPMD 2-slot layout:
    every core runs the same program with two independent weight sets
    (slot A: sA columns, slot B: sB columns); each slot processes a
    contiguous piece of ONE expert's token list, and a hot expert's
    tokens split across slots on different cores. For the observed
    counts this packs 8192 assignments into 8 x (532+504) = 8288
    columns (csum 1036) instead of padding every core to the hottest
    expert (1064).
  - Each slot computes o = relu(x @ W1[e] + b1[e]) @ W2[e] with fp16
    matmuls (fp32 PSUM accumulation).
  - Host combines: out[t] = g1*(o_e1[t] + b2[e1]) + g2*(o_e2[t] + b2[e2]).

fp8 (v3): the last NF8 (8) of the 32 F-subtiles run layer 2 in fp8-e4m3
DoubleRow pairs (PE double-pumped, 0.5 cycles/output-col vs 1.0):
  - L1 for those subtiles is still a full-precision fp16 matmul; its
    evict writes h8 = e4m3(SH*h) (SH=4 pre-folded into W1 columns —
    a power of two, so the fp16 weight scaling is exact).
  - Their W2 rows are quantized host-side to e4m3(SW*W2), SW=16; the
    fp16 W2 rows are scaled by C=SH*SW=64 (exact in fp16), so both
    halves accumulate into one PSUM at scale C; the final evict is a
    single tensor_scalar multiply by 1/C.
  - Measured end-to-end max err vs the fp32 reference: ~1.85e-2
    relative to max|expected| (quantization-noise dominated; the
    dense-fp16 baseline is 5.8e-4).

Layout notes (hand-rolled tile loop):
  - Activations stay feature-major (xT/hT/oT all [128, blk, col]) so both
    layers contract along the partition dim with zero on-device transposes.
  - Weights are pre-tiled on host to [ms, 128ki, k, 128m] so each
    per-m-subtile DMA is one fully contiguous read (2KB+ per partition).
  - Slot A's weight stream rides the SP ring, slot B's the Activation
    ring; xT k-tiles 0-3 go on the GpSimd ring and k4-k7 alternate
    SP/Act, so the early k-tile supply comes from three queues and the
    layer-1 ramp never starves (DMA consumers unblock at the issuing
    queue's slot end, so per-queue serialization is what matters).
  - Four dummy matmuls on a zeroed scratch tile bridge the initial DMA
    wait; empirically 4x128-wide starting right after the DVE memset is
    the sweet spot for the cost model's PE p-state ramp (fewer or
    earlier/narrower dummies regress it).
  - Layer-1 PSUM evict fuses bias-add+relu+convert into hT via a single
    DVE tensor_scalar; layer-2 evict is one DVE tensor_scalar (mult 1/C)
    + an immediate per-(ms,seg) writeback DMA. W2a (fp16) tiles stream
    per (slot, m-subtile) one subtile ahead; the small fp8 W2b tiles
    stay resident. The last layer-2 m-subtile runs piece-major with a
    128-wide final piece, so the post-compute drain is one small evict
    + one floor-cost DMA + completion (~3.2us, mostly the fixed DMA
    init delay + exit barriers).
"""

import numpy as np

import concourse.mybir as mybir
import concourse.tile as tile
from concourse import bacc
from concourse.bass_utils import run_bass_kernel_spmd

B, S, D, F, E = 2, 2048, 1024, 4096, 8
T = B * S
TOP_K = 2
N_CORES = 8
KD = D // 128   # 8  k-tiles in layer 1 / m-subtiles in layer 2
KF = F // 128   # 32 m-subtiles in layer 1 / k-tiles in layer 2

NF8 = 8          # F-subtiles (of KF) whose layer-2 contraction runs in fp8
NP8 = NF8 // 2   # DoubleRow pairs
KF16 = KF - NF8  # fp16 F-subtiles
SH = 4.0         # h pre-scale folded into W1 columns (pow2: exact in fp16)
SW = 16.0        # W2 fp8 quantization scale
CC = SH * SW     # fp16-W2 fold; final evict multiplies PSUM by 1/CC

_build_cache = {}
_last_run = None


def profile_last(trace_cores=None):
    """Re-run the most recent kernel() dispatch with NTFF tracing; returns
    BassKernelResults (exec_time_ns etc.). Dev-harness helper only."""
    nc, in_maps = _last_run
    return run_bass_kernel_spmd(nc, in_maps, list(range(N_CORES)),
                                trace=True, trace_cores=trace_cores)


def _plan_slots(counts):
    """SPMD 2-slot packing: pick slot widths (sA >= sB) and assign each of
    the 16 global slots a contiguous piece of one expert's token list so
    that every expert is fully covered. Returns (sA, sB, assign) with
    assign[core] = ((eA, offA, lenA), (eB, offB, lenB))."""
    counts = [int(c) for c in counts]
    n = len(counts)
    assert n == N_CORES
    maxc = max(counts)
    sA = -(-maxc // 2)
    sA = -(-sA // 4) * 4
    sB = None
    for cand in range(4, sA + 1, 4):
        need = 0
        ok = True
        for c in counts:
            if c <= 2 * cand:
                continue
            if c <= sA + cand:
                need += 1
            elif c <= 2 * sA:
                need += 2
            else:
                ok = False
                break
        if ok and need <= n:
            sB = cand
            break
    assert sB is not None, "2-slot packing infeasible"
    pieces = []  # (len, expert, off, must_big)
    for e, c in enumerate(counts):
        if c <= 2 * sB:
            l1 = -(-c // 2)
            ps = [(l1, False), (c - l1, False)]
        elif c <= sA + sB:
            ps = [(c - sB, True), (sB, False)]
        else:
            l1 = -(-c // 2)
            ps = [(l1, True), (c - l1, True)]
        off = 0
        for ln, big in ps:
            pieces.append((ln, e, off, big))
            off += ln
    bigs = [p for p in pieces if p[3]]
    flex = sorted([p for p in pieces if not p[3]], reverse=True)
    a_slots = bigs + flex[:n - len(bigs)]
    b_slots = flex[n - len(bigs):]
    assert len(a_slots) == n and len(b_slots) == n
    assert all(p[0] <= sA for p in a_slots)
    assert all(p[0] <= sB for p in b_slots)
    assign = [((a[1], a[2], a[0]), (b[1], b[2], b[0]))
              for a, b in zip(a_slots, b_slots)]
    return sA, sB, assign


def _slot_segs(sA, sB):
    """PSUM segments (start, len, slot), each <=512, not crossing the
    slot boundary."""
    segs = []
    for slot, (off, w) in enumerate([(0, sA), (sA, sB)]):
        while w > 0:
            ww = min(512, w)
            segs.append((off, ww, slot))
            off += ww
            w -= ww
    return segs


def _build(sA, sB):
    """Compile the per-core 2-slot expert-MLP kernel (SPMD: same program,
    per-core weights/tokens)."""
    key = (sA, sB)
    if key in _build_cache:
        return _build_cache[key]

    f32 = mybir.dt.float32
    f16 = mybir.dt.float16
    f8 = mybir.dt.float8e4
    csum = sA + sB
    segs = _slot_segs(sA, sB)
    nseg = len(segs)
    csum16 = -(-csum // 16) * 16  # hT8 pair-axis stride, 16B aligned

    nc = bacc.Bacc("TRN2", target_bir_lowering=False, debug=False,
                   num_devices=N_CORES)
    xT = nc.dram_tensor("xT", [128, KD, csum], f16, kind="ExternalInput")
    w1A = nc.dram_tensor("w1A", [KF, 128, KD, 128], f16, kind="ExternalInput")
    w1B = nc.dram_tensor("w1B", [KF, 128, KD, 128], f16, kind="ExternalInput")
    b1A = nc.dram_tensor("b1A", [128, KF], f32, kind="ExternalInput")
    b1B = nc.dram_tensor("b1B", [128, KF], f32, kind="ExternalInput")
    w2aA = nc.dram_tensor("w2aA", [KD, 128, KF16, 128], f16,
                          kind="ExternalInput")
    w2aB = nc.dram_tensor("w2aB", [KD, 128, KF16, 128], f16,
                          kind="ExternalInput")
    w2bA = nc.dram_tensor("w2bA", [KD, 128, NF8, 128], f8,
                          kind="ExternalInput")
    w2bB = nc.dram_tensor("w2bB", [KD, 128, NF8, 128], f8,
                          kind="ExternalInput")
    oT = nc.dram_tensor("oT", [128, KD, csum], f16, kind="ExternalOutput")

    w1_dram = (w1A, w1B)
    w2a_dram = (w2aA, w2aB)
    w2b_dram = (w2bA, w2bB)
    # slot -> issuing ring for its weight stream
    ring = (lambda: nc.sync, lambda: nc.scalar)

    with tile.TileContext(nc) as tc:
        from contextlib import ExitStack
        with ExitStack() as ctx:
            resident = ctx.enter_context(tc.tile_pool(name="resident", bufs=1))
            xT_sb = resident.tile([128, KD, csum], f16)
            hT16 = resident.tile([128, KF16, csum], f16)
            hT8 = resident.tile([128, NF8, csum16], f8)
            oT_sb = resident.tile([128, KD, csum], f16)
            b1_sb = [resident.tile([128, KF], f32, name=f"b1s{s}")
                     for s in range(2)]

            w1_pool = [ctx.enter_context(tc.tile_pool(name=f"w1p{s}", bufs=4))
                       for s in range(2)]
            w2a_pool = ctx.enter_context(tc.tile_pool(name="w2a", bufs=4))
            w2b_pool = ctx.enter_context(tc.tile_pool(name="w2b", bufs=1))
            # fp8 W2 tiles are small (1KB/partition): keep all resident
            w2bt = [[w2b_pool.tile([128, NF8, 128], f8, name=f"w2b{s}_{ms}")
                     for ms in range(KD)] for s in range(2)]
            psum = ctx.enter_context(
                tc.tile_pool(name="psum", bufs=2, space="PSUM"))

            def psum_group():
                return [psum.tile([128, 512], f32, name=f"p{si}",
                                  tag=f"p{si}")
                        for si in range(nseg)]

            # PE pre-warm: the first real matmul can't start until its
            # input DMAs complete (~1.9us). Matmuls on a zeroed scratch
            # tile keep the PE p-state ramp running through that window.
            if 2 * nseg + 1 <= 8:
                warm_sb = resident.tile([128, 256], f16)
                nc.vector.memset(warm_sb[:], 0.0)
                warm_ps = psum.tile([128, 512], f32, name="warm", tag="warm",
                                    bufs=1)
                for _ in range(4):
                    nc.tensor.matmul(warm_ps[:, :128], lhsT=warm_sb[:, :128],
                                     rhs=warm_sb[:, 128:256],
                                     start=True, stop=True)

            # DMA preamble. SP ring: slot-A w1 stream + xT k4-7; Act ring:
            # slot-B w1 stream + xT k0-3 + biases. Interleave so the first
            # few weight tiles of both slots land before the PE needs them.
            half = KD // 2
            w1_first = [[], []]
            t = w1_pool[0].tile([128, KD, 128], f16, name="w1tA")
            nc.sync.dma_start(t[:], w1A.ap()[0])
            w1_first[0].append(t)
            t = w1_pool[1].tile([128, KD, 128], f16, name="w1tB")
            nc.scalar.dma_start(t[:], w1B.ap()[0])
            w1_first[1].append(t)
            nc.gpsimd.dma_start(xT_sb[:, 0, :sA], xT.ap()[:, 0, :sA])
            nc.gpsimd.dma_start(xT_sb[:, 0, sA:], xT.ap()[:, 0, sA:])
            for i in range(1, half):
                nc.gpsimd.dma_start(xT_sb[:, i, :], xT.ap()[:, i, :])
            for i in range(half, KD):
                (nc.sync if i % 2 == 0 else nc.scalar).dma_start(
                    xT_sb[:, i, :], xT.ap()[:, i, :])
            for i in range(1, 4):
                t = w1_pool[0].tile([128, KD, 128], f16, name="w1tA")
                nc.sync.dma_start(t[:], w1A.ap()[i])
                w1_first[0].append(t)
                t = w1_pool[1].tile([128, KD, 128], f16, name="w1tB")
                nc.scalar.dma_start(t[:], w1B.ap()[i])
                w1_first[1].append(t)
            nc.scalar.dma_start(b1_sb[0][:], b1A.ap())
            nc.scalar.dma_start(b1_sb[1][:], b1B.ap())

            # ---- layer 1: h = relu(W1.T @ xT + b1) (SBUF-resident) ----
            # fp16-destined m-subtiles evict to hT16; fp8-destined ones
            # (ms >= KF16, W1 columns pre-scaled by SH) evict to hT8.
            w2b_dma_after = {6 + 3 * j: j for j in range(KD)}
            for ms in range(KF):
                w1t = []
                for s in range(2):
                    if ms < len(w1_first[s]):
                        w1t.append(w1_first[s][ms])
                    else:
                        t = w1_pool[s].tile([128, KD, 128], f16,
                                            name=("w1tA", "w1tB")[s])
                        ring[s]().dma_start(t[:], w1_dram[s].ap()[ms])
                        w1t.append(t)
                if ms in w2b_dma_after:
                    j = w2b_dma_after[ms]
                    nc.sync.dma_start(w2bt[0][j][:], w2bA.ap()[j])
                    nc.scalar.dma_start(w2bt[1][j][:], w2bB.ap()[j])
                pts = psum_group()
                for k in range(KD):
                    for si, (st, ln, sl) in enumerate(segs):
                        nc.tensor.matmul(pts[si][:, :ln],
                                         lhsT=w1t[sl][:, k, :],
                                         rhs=xT_sb[:, k, st:st + ln],
                                         start=(k == 0),
                                         stop=(k == KD - 1))
                for si, (st, ln, sl) in enumerate(segs):
                    if ms < KF16:
                        dst = hT16[:, ms, st:st + ln]
                    else:
                        dst = hT8[:, ms - KF16, st:st + ln]
                    nc.vector.tensor_scalar(
                        dst, pts[si][:, :ln],
                        b1_sb[sl][:, ms:ms + 1], 0.0,
                        mybir.AluOpType.add, mybir.AluOpType.max)

            # ---- layer 2: oT = (W2a.T @ hT16 + W2b.T @ hT8) / CC ----
            # KF16 fp16 k-tiles + NP8 fp8 DoubleRow pairs accumulate into
            # one PSUM group; evict scales by 1/CC. W2a tiles stream
            # per (slot, ms), one m-subtile ahead.
            def w2a_fetch(ms):
                ts = []
                for s in range(2):
                    t = w2a_pool.tile([128, KF16, 128], f16,
                                      name=("w2aA", "w2aB")[s])
                    ring[s]().dma_start(t[:], w2a_dram[s].ap()[ms])
                    ts.append(t)
                return ts

            def l2_mms(pt, w2at, ms, st, ln, sl):
                for k in range(KF16):
                    nc.tensor.matmul(pt[:, :ln],
                                     lhsT=w2at[sl][:, k, :],
                                     rhs=hT16[:, k, st:st + ln],
                                     start=(k == 0), stop=False)
                for j in range(NP8):
                    nc.tensor.matmul(
                        pt[:, :ln],
                        lhsT=w2bt[sl][ms][:, 2 * j:2 * j + 2, :],
                        rhs=hT8[:, 2 * j:2 * j + 2, st:st + ln],
                        perf_mode=mybir.MatmulPerfMode.DoubleRow,
                        start=False, stop=(j == NP8 - 1))

            inv_c = 1.0 / CC
            w2a_cur = w2a_fetch(0)
            for ms in range(KD):
                w2a_nxt = w2a_fetch(ms + 1) if ms + 1 < KD else None
                if ms < KD - 1:
                    pts = psum_group()
                    for k in range(KF16):
                        for si, (st, ln, sl) in enumerate(segs):
                            nc.tensor.matmul(pts[si][:, :ln],
                                             lhsT=w2a_cur[sl][:, k, :],
                                             rhs=hT16[:, k, st:st + ln],
                                             start=(k == 0), stop=False)
                    for j in range(NP8):
                        for si, (st, ln, sl) in enumerate(segs):
                            nc.tensor.matmul(
                                pts[si][:, :ln],
                                lhsT=w2bt[sl][ms][:, 2 * j:2 * j + 2, :],
                                rhs=hT8[:, 2 * j:2 * j + 2, st:st + ln],
                                perf_mode=mybir.MatmulPerfMode.DoubleRow,
                                start=False, stop=(j == NP8 - 1))
                    for si, (st, ln, sl) in enumerate(segs):
                        nc.vector.tensor_scalar(
                            oT_sb[:, ms, st:st + ln], pts[si][:, :ln],
                            inv_c, None, mybir.AluOpType.mult)
                        nc.sync.dma_start(oT.ap()[:, ms, st:st + ln],
                                          oT_sb[:, ms, st:st + ln])
                else:
                    # piece-major with a small final piece: each piece's
                    # evict+writeback hides under the next piece's matmuls,
                    # so the post-compute drain is only the final piece's
                    # evict + DMA + completion.
                    small = 128
                    lsegs = sorted(segs[:-1], key=lambda s: s[1])
                    lst, lln, lsl = segs[-1]
                    if lln > small:
                        lsegs.append((lst, lln - small, lsl))
                        lsegs.append((lst + lln - small, small, lsl))
                    else:
                        lsegs.append(segs[-1])
                    for si, (st, ln, sl) in enumerate(lsegs):
                        pt = psum.tile([128, 512], f32,
                                       name=f"p{si % nseg}",
                                       tag=f"p{si % nseg}")
                        l2_mms(pt, w2a_cur, ms, st, ln, sl)
                        nc.vector.tensor_scalar(
                            oT_sb[:, ms, st:st + ln], pt[:, :ln],
                            inv_c, None, mybir.AluOpType.mult)
                        nc.sync.dma_start(oT.ap()[:, ms, st:st + ln],
                                          oT_sb[:, ms, st:st + ln])
                if w2a_nxt is not None:
                    w2a_cur = w2a_nxt

    nc.compile()
    _build_cache[key] = nc
    return nc


def _route(x2d, noise2d, Wr, br, Wn, bn):
    """Noisy top-2 router in float64. Returns (top2 ids [T,2], gates [T,2])."""
    x64 = x2d.astype(np.float64)
    logits = x64 @ Wr.astype(np.float64) + br.astype(np.float64)
    nl = x64 @ Wn.astype(np.float64) + bn.astype(np.float64)
    noisy = logits + noise2d.astype(np.float64) * np.logaddexp(0.0, nl)
    # stable argsort of -noisy == jax.lax.top_k tie-breaking (lower index wins)
    top2 = np.argsort(-noisy, axis=-1, kind="stable")[:, :TOP_K]
    v = np.take_along_axis(noisy, top2, axis=-1)
    v = v - v.max(axis=-1, keepdims=True)
    ev = np.exp(v)
    gates = ev / ev.sum(axis=-1, keepdims=True)
    return top2, gates


def kernel(x, noise, Wr, br, Wn, bn, W1, b1, W2, b2):
    import ml_dtypes
    e4m3 = ml_dtypes.float8_e4m3fn

    x = np.ascontiguousarray(np.asarray(x, dtype=np.float32))
    x2d = x.reshape(T, D)
    top2, gates = _route(x2d, np.asarray(noise).reshape(T, E),
                         np.asarray(Wr), np.asarray(br),
                         np.asarray(Wn), np.asarray(bn))

    # dispatch: stable sort of the 2T assignments by expert id
    expert_ids = top2.ravel()  # assignment a -> expert; token = a // 2
    ord_ = np.argsort(expert_ids, kind="stable")
    counts = np.bincount(expert_ids, minlength=E)
    starts = np.zeros(E + 1, dtype=np.int64)
    np.cumsum(counts, out=starts[1:])

    sA, sB, assign = _plan_slots(counts)
    csum = sA + sB
    nc = _build(sA, sB)

    W1 = np.asarray(W1, dtype=np.float32)
    W2 = np.asarray(W2, dtype=np.float32)
    b1 = np.asarray(b1, dtype=np.float32)
    b2 = np.asarray(b2, dtype=np.float32)
    x16 = x2d.astype(np.float16)

    # fp8 scale folding (SH, CC are powers of two: exact in fp16)
    colscale = np.ones(F, dtype=np.float32)
    colscale[KF16 * 128:] = SH
    FS = KF16 * 128  # F split row

    # per-expert prepped weights (shared across slots referencing the
    # same expert)
    w1_t, b1_t, w2a_t, w2b_t = {}, {}, {}, {}

    def prep(e):
        if e in w1_t:
            return
        w1_t[e] = np.ascontiguousarray(
            (W1[e] * colscale).astype(np.float16).reshape(KD, 128, KF, 128)
            .transpose(2, 1, 0, 3))
        b1_t[e] = np.ascontiguousarray(
            (b1[e] * colscale).reshape(KF, 128).T)
        w2a_t[e] = np.ascontiguousarray(
            (W2[e][:FS] * CC).astype(np.float16).reshape(KF16, 128, KD, 128)
            .transpose(2, 1, 0, 3))
        w2b_t[e] = np.ascontiguousarray(
            np.clip(W2[e][FS:] * SW, -224, 224).astype(e4m3)
            .reshape(NF8, 128, KD, 128).transpose(2, 1, 0, 3))

    in_maps = []
    for core in range(N_CORES):
        (eA, offA, lA), (eB, offB, lB) = assign[core]
        prep(eA)
        prep(eB)
        xe = np.zeros((csum, D), dtype=np.float16)
        toksA = ord_[starts[eA] + offA:starts[eA] + offA + lA] // 2
        toksB = ord_[starts[eB] + offB:starts[eB] + offB + lB] // 2
        xe[:lA] = x16[toksA]
        xe[sA:sA + lB] = x16[toksB]
        # xT[ki, k, c] = xe[c, k*128+ki]
        xTe = np.ascontiguousarray(
            xe.T.reshape(KD, 128, csum).transpose(1, 0, 2))
        in_maps.append({
            "xT": xTe,
            "w1A": w1_t[eA], "w1B": w1_t[eB],
            "b1A": b1_t[eA], "b1B": b1_t[eB],
            "w2aA": w2a_t[eA], "w2aB": w2a_t[eB],
            "w2bA": w2b_t[eA], "w2bB": w2b_t[eB],
        })

    res = None
    for attempt in range(3):
        try:
            res = run_bass_kernel_spmd(nc, in_maps, list(range(N_CORES)))
            break
        except Exception:
            if attempt == 2:
                raise
            import time
            time.sleep(5)
    global _last_run
    _last_run = (nc, in_maps)

    # combine: A holds expert outputs in assignment-sorted order
    Aacc = np.empty((2 * T, D), dtype=np.float32)
    pos = np.empty(2 * T, dtype=np.int64)
    pos[ord_] = np.arange(2 * T)
    for core in range(N_CORES):
        oTe = res.results[core]["oT"]  # [128, KD, csum] f16
        (eA, offA, lA), (eB, offB, lB) = assign[core]
        for (e, off, ln, c0) in ((eA, offA, lA, 0), (eB, offB, lB, sA)):
            if ln == 0:
                continue
            oe = oTe[:, :, c0:c0 + ln].transpose(2, 1, 0).reshape(ln, D)
            Aacc[starts[e] + off:starts[e] + off + ln] = \
                oe.astype(np.float32) + b2[e]
    out = (gates[:, :, None] *
           Aacc[pos.reshape(T, TOP_K)].astype(np.float64)).sum(axis=1)
    return out.reshape(B, S, D).astype(np.float32)


# revision 20
# speedup vs baseline: 1.0071x; 1.0071x over previous
"""MoE (noisy top-2 routing, 8 experts) on 8 Trainium2 NeuronCores.

Strategy (expert-parallel, per the sharding hint):
  - Router runs on host in float64 (134 MFLOP — negligible next to the
    137 GFLOP expert MLPs; fp64 makes the top-k selection robust).
  - Tokens are dispatched by top-2 expert id. v5 balances the per-core
    column count with an SPMD 3-slot layout: every core runs the same
    program with three independent weight sets (slot widths found by a
    small exact search over slot sizes + a memoized cover of the expert
    counts). For the observed counts this packs the 8192 assignments
    into 8 x (504+492+32) = 8224 columns (csum 1028) instead of padding
    every core to the hottest expert (1064).
  - Each slot computes o = relu(x @ W1[e] + b1[e]) @ W2[e] with fp16
    matmuls (fp32 PSUM accumulation).
  - Host combines: out[t] = g1*(o_e1[t] + b2[e1]) + g2*(o_e2[t] + b2[e2]).

fp8: the last NF8 (8) of the 32 F-subtiles run layer 2 in fp8-e4m3
DoubleRow pairs (PE double-pumped, 0.5 cycles/output-col vs 1.0):
  - L1 for those subtiles is still a full-precision fp16 matmul; its
    evict writes h8 = e4m3(SH*h) (SH=4 pre-folded into W1 columns —
    a power of two, so the fp16 weight scaling is exact).
  - Their W2 rows are quantized host-side to e4m3(SW*W2), SW=16; the
    fp16 W2 rows are scaled by C=SH*SW=64 (exact in fp16), so both
    halves accumulate into one PSUM at scale C; the final evict is a
    single tensor_scalar multiply by 1/C.
  - Measured end-to-end max err vs the fp32 reference: ~1.85e-2
    relative to max|expected| (quantization-noise dominated; the
    dense-fp16 baseline is 5.8e-4).

Layout notes (hand-rolled tile loop):
  - Activations stay feature-major (xT/hT/oT all [128, blk, col]) so both
    layers contract along the partition dim with zero on-device transposes.
  - Weights are pre-tiled on host to [ms, 128ki, k, 128m] so each
    per-m-subtile DMA is one fully contiguous read (1KB+ per partition).
  - One DMA ring per slot weight stream: SP, Activation, GpSimd. xT
    k-tiles 0-3 lead the GpSimd ring and k4-k7 alternate SP/Act, so the
    early k-tile supply comes from three queues (DMA consumers unblock
    at the issuing queue's slot end, so per-queue serialization is what
    matters). Slot 2's matmuls run LAST within each m-subtile, giving
    its (GpSimd) weight stream ~6us of slack behind the xT tiles.
  - Four dummy matmuls on a zeroed scratch tile bridge the initial DMA
    wait; empirically 4x128-wide starting right after the DVE memset is
    the sweet spot for the cost model's PE p-state ramp.
  - Layer-1 PSUM evict fuses bias-add+relu+convert into hT via a single
    DVE tensor_scalar; layer-2 evict is one DVE tensor_scalar (mult 1/C)
    + an immediate per-(ms,seg) writeback DMA. W2a (fp16) tiles stream
    per (slot, m-subtile) one subtile ahead; the small fp8 W2b tiles
    stay resident. The last layer-2 m-subtile runs piece-major with a
    128-wide final piece, so the post-compute drain is one small evict
    + one floor-cost DMA + completion (~3.2us, mostly the fixed DMA
    init delay + exit barriers).
"""

import itertools
from functools import lru_cache

import numpy as np

import concourse.mybir as mybir
import concourse.tile as tile
from concourse import bacc
from concourse.bass_utils import run_bass_kernel_spmd

B, S, D, F, E = 2, 2048, 1024, 4096, 8
T = B * S
TOP_K = 2
N_CORES = 8
NSLOT = 3
KD = D // 128   # 8  k-tiles in layer 1 / m-subtiles in layer 2
KF = F // 128   # 32 m-subtiles in layer 1 / k-tiles in layer 2

NF8 = 8          # F-subtiles (of KF) whose layer-2 contraction runs in fp8
NP8 = NF8 // 2   # DoubleRow pairs
KF16 = KF - NF8  # fp16 F-subtiles
SH = 4.0         # h pre-scale folded into W1 columns (pow2: exact in fp16)
SW = 16.0        # W2 fp8 quantization scale
CC = SH * SW     # fp16-W2 fold; final evict multiplies PSUM by 1/CC

_build_cache = {}
_last_run = None


def profile_last(trace_cores=None):
    """Re-run the most recent kernel() dispatch with NTFF tracing; returns
    BassKernelResults (exec_time_ns etc.). Dev-harness helper only."""
    nc, in_maps = _last_run
    return run_bass_kernel_spmd(nc, in_maps, list(range(N_CORES)),
                                trace=True, trace_cores=trace_cores)


def _cover(sizes, counts, max_slots):
    """Assign each expert a disjoint multiset of slots (8 of each size)
    with capacity >= count. Returns per-expert slot-type vectors or None."""
    order = sorted(range(len(counts)), key=lambda e: -counts[e])
    cs = [counts[e] for e in order]
    opts_cache = {}

    def options(c):
        if c in opts_cache:
            return opts_cache[c]
        outs = []
        for a in itertools.product(*[range(0, min(8, max_slots) + 1)
                                     for _ in sizes]):
            n = sum(a)
            if n == 0 or n > max_slots:
                continue
            cap = sum(x * s for x, s in zip(a, sizes))
            if cap >= c:
                outs.append((cap - c, a))
        outs.sort()
        opts_cache[c] = outs[:60]
        return outs[:60]

    @lru_cache(maxsize=None)
    def dfs(i, sup):
        if i == len(cs):
            return ()
        for _, a in options(cs[i]):
            ns = tuple(s - x for s, x in zip(sup, a))
            if min(ns) < 0:
                continue
            rest = dfs(i + 1, ns)
            if rest is not None:
                return (a,) + rest
        return None

    sol = dfs(0, (8,) * len(sizes))
    if sol is None:
        return None
    out = [None] * len(counts)
    for idx, e in enumerate(order):
        out[e] = sol[idx]
    return out


def _plan_slots(counts):
    """SPMD 3-slot packing: pick slot widths (s0 >= s1 >= s2, each <=512)
    and cover the expert counts with the 24 global slots. Returns
    (sizes, assign) with assign[core] = tuple of NSLOT (e, off, len)."""
    counts = [int(c) for c in counts]
    n = len(counts)
    assert n == N_CORES
    total = sum(counts)
    lo = max(-(-total // n), -(-max(counts) // 4))
    lo = -(-lo // 4) * 4
    best = None
    for C in range(lo, 2 * max(max(counts), 512) + 1, 4):
        for s0 in range(min(C - 8, 512), (C + NSLOT - 1) // NSLOT - 1, -4):
            for s1 in range(min(s0, C - s0 - 4), 0, -4):
                s2 = C - s0 - s1
                if s2 < 4 or s2 > s1:
                    continue
                alloc = _cover((s0, s1, s2), counts, max_slots=4)
                if alloc is not None:
                    best = ((s0, s1, s2), alloc)
                    break
            if best:
                break
        if best:
            break
    assert best is not None, "slot packing infeasible"
    sizes, alloc = best

    # pieces per expert, filling its largest slots first so the remainder
    # lands in the smallest assigned slot
    per_type = [[] for _ in range(NSLOT)]  # lists of (expert, off, len)
    for e, a in enumerate(alloc):
        rem = counts[e]
        off = 0
        for ti in range(NSLOT):
            for _ in range(a[ti]):
                ln = min(rem, sizes[ti])
                per_type[ti].append((e, off, ln))
                off += ln
                rem -= ln
        assert rem == 0
    for ti in range(NSLOT):
        while len(per_type[ti]) < n:
            per_type[ti].append((0, 0, 0))
        assert len(per_type[ti]) == n
    assign = [tuple(per_type[ti][i] for ti in range(NSLOT))
              for i in range(n)]
    return sizes, assign


def _build(sizes):
    """Compile the per-core 3-slot expert-MLP kernel (SPMD: same program,
    per-core weights/tokens)."""
    key = tuple(sizes)
    if key in _build_cache:
        return _build_cache[key]

    f32 = mybir.dt.float32
    f16 = mybir.dt.float16
    f8 = mybir.dt.float8e4
    csum = sum(sizes)
    offs = [sum(sizes[:i]) for i in range(NSLOT)]
    segs = [(offs[s], sizes[s], s) for s in range(NSLOT)]
    nseg = len(segs)
    csum16 = -(-csum // 16) * 16  # hT8 pair-axis stride, 16B aligned

    nc = bacc.Bacc("TRN2", target_bir_lowering=False, debug=False,
                   num_devices=N_CORES)
    xT = nc.dram_tensor("xT", [128, KD, csum], f16, kind="ExternalInput")
    w1_dram = [nc.dram_tensor(f"w1_{s}", [KF, 128, KD, 128], f16,
                              kind="ExternalInput") for s in range(NSLOT)]
    b1_dram = [nc.dram_tensor(f"b1_{s}", [128, KF], f32,
                              kind="ExternalInput") for s in range(NSLOT)]
    w2a_dram = [nc.dram_tensor(f"w2a_{s}", [KD, 128, KF16, 128], f16,
                               kind="ExternalInput") for s in range(NSLOT)]
    w2b_dram = [nc.dram_tensor(f"w2b_{s}", [KD, 128, NF8, 128], f8,
                               kind="ExternalInput") for s in range(NSLOT)]
    oT = nc.dram_tensor("oT", [128, KD, csum], f16, kind="ExternalOutput")

    # slot -> issuing ring for its weight stream
    ring = (lambda: nc.sync, lambda: nc.scalar, lambda: nc.gpsimd)

    with tile.TileContext(nc) as tc:
        from contextlib import ExitStack
        with ExitStack() as ctx:
            resident = ctx.enter_context(tc.tile_pool(name="resident", bufs=1))
            xT_sb = resident.tile([128, KD, csum], f16)
            hT16 = resident.tile([128, KF16, csum], f16)
            hT8 = resident.tile([128, NF8, csum16], f8)
            oT_sb = resident.tile([128, KD, csum], f16)
            b1_sb = [resident.tile([128, KF], f32, name=f"b1s{s}")
                     for s in range(NSLOT)]

            w1_pool = [ctx.enter_context(tc.tile_pool(name=f"w1p{s}", bufs=4))
                       for s in range(NSLOT)]
            w2a_pool = ctx.enter_context(tc.tile_pool(name="w2a", bufs=2))
            w2b_pool = ctx.enter_context(tc.tile_pool(name="w2b", bufs=1))
            # fp8 W2 tiles are small (1KB/partition): keep all resident
            w2bt = [[w2b_pool.tile([128, NF8, 128], f8, name=f"w2b{s}_{ms}")
                     for ms in range(KD)] for s in range(NSLOT)]
            psum = ctx.enter_context(
                tc.tile_pool(name="psum", bufs=2, space="PSUM"))

            def psum_group():
                return [psum.tile([128, 512], f32, name=f"p{si}",
                                  tag=f"p{si}")
                        for si in range(nseg)]

            # PE pre-warm: the first real matmul can't start until its
            # input DMAs complete (~1.05us). Dummy matmuls on a zeroed
            # scratch tile keep the PE p-state ramp running to exactly
            # that point (fewer or narrower dummies trip a ramp reset).
            if 2 * nseg + 1 <= 8:
                warm_sb = resident.tile([128, 256], f16)
                nc.vector.memset(warm_sb[:], 0.0)
                warm_ps = psum.tile([128, 512], f32, name="warm", tag="warm",
                                    bufs=1)
                for _ in range(4):
                    nc.tensor.matmul(warm_ps[:, :128], lhsT=warm_sb[:, :128],
                                     rhs=warm_sb[:, 128:256],
                                     start=True, stop=True)

            # DMA preamble. SP: slot-0 w1 stream + xk4/xk6; Act: slot-1
            # w1 stream + xk5/xk7 + biases; GpSimd: xk0-3 then the slot-2
            # w1 stream (slot-2 matmuls run last per m-subtile, so its
            # stream has slack).
            w1_first = [[], [], []]
            t = w1_pool[0].tile([128, KD, 128], f16, name="w1t0")
            nc.sync.dma_start(t[:], w1_dram[0].ap()[0])
            w1_first[0].append(t)
            t = w1_pool[1].tile([128, KD, 128], f16, name="w1t1")
            nc.scalar.dma_start(t[:], w1_dram[1].ap()[0])
            w1_first[1].append(t)
            for i in range(3):
                nc.gpsimd.dma_start(xT_sb[:, i, :], xT.ap()[:, i, :])
            nc.sync.dma_start(xT_sb[:, 3, :], xT.ap()[:, 3, :])
            for i in range(4, KD):
                (nc.sync if i % 2 == 0 else nc.scalar).dma_start(
                    xT_sb[:, i, :], xT.ap()[:, i, :])
            t = w1_pool[2].tile([128, KD, 128], f16, name="w1t2")
            nc.gpsimd.dma_start(t[:], w1_dram[2].ap()[0])
            w1_first[2].append(t)
            for i in range(1, 4):
                for s in range(NSLOT):
                    t = w1_pool[s].tile([128, KD, 128], f16, name=f"w1t{s}")
                    ring[s]().dma_start(t[:], w1_dram[s].ap()[i])
                    w1_first[s].append(t)
            for s in range(NSLOT):
                nc.scalar.dma_start(b1_sb[s][:], b1_dram[s].ap())

            # ---- layer 1: h = relu(W1.T @ xT + b1) (SBUF-resident) ----
            # Slots 0/1 matmul per k first; the narrow slot 2 runs after,
            # so its weight tile may arrive late without stalling the PE.
            # fp16-destined m-subtiles evict to hT16; fp8-destined ones
            # (ms >= KF16, W1 columns pre-scaled by SH) evict to hT8.
            w2b_dma_after = {6 + 3 * j: j for j in range(KD)}
            for ms in range(KF):
                w1t = []
                for s in range(NSLOT):
                    if ms < len(w1_first[s]):
                        w1t.append(w1_first[s][ms])
                    else:
                        t = w1_pool[s].tile([128, KD, 128], f16,
                                            name=f"w1t{s}")
                        ring[s]().dma_start(t[:], w1_dram[s].ap()[ms])
                        w1t.append(t)
                if ms in w2b_dma_after:
                    j = w2b_dma_after[ms]
                    for s in range(NSLOT):
                        ring[s]().dma_start(w2bt[s][j][:], w2b_dram[s].ap()[j])
                pts = psum_group()
                for k in range(KD):
                    for si, (st, ln, sl) in enumerate(segs[:2]):
                        nc.tensor.matmul(pts[si][:, :ln],
                                         lhsT=w1t[sl][:, k, :],
                                         rhs=xT_sb[:, k, st:st + ln],
                                         start=(k == 0),
                                         stop=(k == KD - 1))
                st, ln, sl = segs[2]
                for k in range(KD):
                    nc.tensor.matmul(pts[2][:, :ln],
                                     lhsT=w1t[sl][:, k, :],
                                     rhs=xT_sb[:, k, st:st + ln],
                                     start=(k == 0),
                                     stop=(k == KD - 1))
                for si, (st, ln, sl) in enumerate(segs):
                    if ms < KF16:
                        dst = hT16[:, ms, st:st + ln]
                    else:
                        dst = hT8[:, ms - KF16, st:st + ln]
                    nc.vector.tensor_scalar(
                        dst, pts[si][:, :ln],
                        b1_sb[sl][:, ms:ms + 1], 0.0,
                        mybir.AluOpType.add, mybir.AluOpType.max)

            # ---- layer 2: oT = (W2a.T @ hT16 + W2b.T @ hT8) / CC ----
            # KF16 fp16 k-tiles + NP8 fp8 DoubleRow pairs accumulate into
            # one PSUM group; evict scales by 1/CC. W2a tiles stream
            # per (slot, ms), one m-subtile ahead, each on its slot ring.
            def w2a_fetch(ms):
                ts = []
                for s in range(NSLOT):
                    t = w2a_pool.tile([128, KF16, 128], f16, name=f"w2a{s}")
                    ring[s]().dma_start(t[:], w2a_dram[s].ap()[ms])
                    ts.append(t)
                return ts

            def l2_seg_mms(pt, w2at, ms, st, ln, sl):
                for k in range(KF16):
                    nc.tensor.matmul(pt[:, :ln],
                                     lhsT=w2at[sl][:, k, :],
                                     rhs=hT16[:, k, st:st + ln],
                                     start=(k == 0), stop=False)
                for j in range(NP8):
                    nc.tensor.matmul(
                        pt[:, :ln],
                        lhsT=w2bt[sl][ms][:, 2 * j:2 * j + 2, :],
                        rhs=hT8[:, 2 * j:2 * j + 2, st:st + ln],
                        perf_mode=mybir.MatmulPerfMode.DoubleRow,
                        start=False, stop=(j == NP8 - 1))

            inv_c = 1.0 / CC
            w2a_cur = w2a_fetch(0)
            for ms in range(KD):
                w2a_nxt = w2a_fetch(ms + 1) if ms + 1 < KD else None
                if ms < KD - 1:
                    pts = psum_group()
                    for k in range(KF16):
                        for si, (st, ln, sl) in enumerate(segs[:2]):
                            nc.tensor.matmul(pts[si][:, :ln],
                                             lhsT=w2a_cur[sl][:, k, :],
                                             rhs=hT16[:, k, st:st + ln],
                                             start=(k == 0), stop=False)
                    for j in range(NP8):
                        for si, (st, ln, sl) in enumerate(segs[:2]):
                            nc.tensor.matmul(
                                pts[si][:, :ln],
                                lhsT=w2bt[sl][ms][:, 2 * j:2 * j + 2, :],
                                rhs=hT8[:, 2 * j:2 * j + 2, st:st + ln],
                                perf_mode=mybir.MatmulPerfMode.DoubleRow,
                                start=False, stop=(j == NP8 - 1))
                    st, ln, sl = segs[2]
                    l2_seg_mms(pts[2], w2a_cur, ms, st, ln, sl)
                    for si, (st, ln, sl) in enumerate(segs):
                        nc.vector.tensor_scalar(
                            oT_sb[:, ms, st:st + ln], pts[si][:, :ln],
                            inv_c, None, mybir.AluOpType.mult)
                        nc.sync.dma_start(oT.ap()[:, ms, st:st + ln],
                                          oT_sb[:, ms, st:st + ln])
                else:
                    # piece-major with a small final piece: each piece's
                    # evict+writeback hides under the next piece's matmuls,
                    # so the post-compute drain is only the final piece's
                    # evict + DMA + completion.
                    small = 128
                    lsegs = sorted(segs, key=lambda s: s[1])
                    lst, lln, lsl = lsegs[-1]
                    if lln > small:
                        lsegs = lsegs[:-1] + [(lst, lln - small, lsl),
                                              (lst + lln - small, small, lsl)]
                    for si, (st, ln, sl) in enumerate(lsegs):
                        pt = psum.tile([128, 512], f32,
                                       name=f"p{si % nseg}",
                                       tag=f"p{si % nseg}")
                        l2_seg_mms(pt, w2a_cur, ms, st, ln, sl)
                        nc.vector.tensor_scalar(
                            oT_sb[:, ms, st:st + ln], pt[:, :ln],
                            inv_c, None, mybir.AluOpType.mult)
                        nc.sync.dma_start(oT.ap()[:, ms, st:st + ln],
                                          oT_sb[:, ms, st:st + ln])
                if w2a_nxt is not None:
                    w2a_cur = w2a_nxt

    nc.compile()
    _build_cache[key] = nc
    return nc


def _route(x2d, noise2d, Wr, br, Wn, bn):
    """Noisy top-2 router in float64. Returns (top2 ids [T,2], gates [T,2])."""
    x64 = x2d.astype(np.float64)
    logits = x64 @ Wr.astype(np.float64) + br.astype(np.float64)
    nl = x64 @ Wn.astype(np.float64) + bn.astype(np.float64)
    noisy = logits + noise2d.astype(np.float64) * np.logaddexp(0.0, nl)
    # stable argsort of -noisy == jax.lax.top_k tie-breaking (lower index wins)
    top2 = np.argsort(-noisy, axis=-1, kind="stable")[:, :TOP_K]
    v = np.take_along_axis(noisy, top2, axis=-1)
    v = v - v.max(axis=-1, keepdims=True)
    ev = np.exp(v)
    gates = ev / ev.sum(axis=-1, keepdims=True)
    return top2, gates


def kernel(x, noise, Wr, br, Wn, bn, W1, b1, W2, b2):
    import ml_dtypes
    e4m3 = ml_dtypes.float8_e4m3fn

    x = np.ascontiguousarray(np.asarray(x, dtype=np.float32))
    x2d = x.reshape(T, D)
    top2, gates = _route(x2d, np.asarray(noise).reshape(T, E),
                         np.asarray(Wr), np.asarray(br),
                         np.asarray(Wn), np.asarray(bn))

    # dispatch: stable sort of the 2T assignments by expert id
    expert_ids = top2.ravel()  # assignment a -> expert; token = a // 2
    ord_ = np.argsort(expert_ids, kind="stable")
    counts = np.bincount(expert_ids, minlength=E)
    starts = np.zeros(E + 1, dtype=np.int64)
    np.cumsum(counts, out=starts[1:])

    sizes, assign = _plan_slots(counts)
    csum = sum(sizes)
    offs = [sum(sizes[:i]) for i in range(NSLOT)]
    nc = _build(sizes)

    W1 = np.asarray(W1, dtype=np.float32)
    W2 = np.asarray(W2, dtype=np.float32)
    b1 = np.asarray(b1, dtype=np.float32)
    b2 = np.asarray(b2, dtype=np.float32)
    x16 = x2d.astype(np.float16)

    # fp8 scale folding (SH, CC are powers of two: exact in fp16)
    colscale = np.ones(F, dtype=np.float32)
    colscale[KF16 * 128:] = SH
    FS = KF16 * 128  # F split row

    # per-expert prepped weights (shared across slots referencing the
    # same expert)
    w1_t, b1_t, w2a_t, w2b_t = {}, {}, {}, {}

    def prep(e):
        if e in w1_t:
            return
        w1_t[e] = np.ascontiguousarray(
            (W1[e] * colscale).astype(np.float16).reshape(KD, 128, KF, 128)
            .transpose(2, 1, 0, 3))
        b1_t[e] = np.ascontiguousarray(
            (b1[e] * colscale).reshape(KF, 128).T)
        w2a_t[e] = np.ascontiguousarray(
            (W2[e][:FS] * CC).astype(np.float16).reshape(KF16, 128, KD, 128)
            .transpose(2, 1, 0, 3))
        w2b_t[e] = np.ascontiguousarray(
            np.clip(W2[e][FS:] * SW, -224, 224).astype(e4m3)
            .reshape(NF8, 128, KD, 128).transpose(2, 1, 0, 3))

    in_maps = []
    for core in range(N_CORES):
        xe = np.zeros((csum, D), dtype=np.float16)
        im = {}
        for s in range(NSLOT):
            e, off, ln = assign[core][s]
            prep(e)
            toks = ord_[starts[e] + off:starts[e] + off + ln] // 2
            xe[offs[s]:offs[s] + ln] = x16[toks]
            im[f"w1_{s}"] = w1_t[e]
            im[f"b1_{s}"] = b1_t[e]
            im[f"w2a_{s}"] = w2a_t[e]
            im[f"w2b_{s}"] = w2b_t[e]
        # xT[ki, k, c] = xe[c, k*128+ki]
        im["xT"] = np.ascontiguousarray(
            xe.T.reshape(KD, 128, csum).transpose(1, 0, 2))
        in_maps.append(im)

    res = None
    for attempt in range(3):
        try:
            res = run_bass_kernel_spmd(nc, in_maps, list(range(N_CORES)))
            break
        except Exception:
            if attempt == 2:
                raise
            import time
            time.sleep(5)
    global _last_run
    _last_run = (nc, in_maps)

    # combine: Aacc holds expert outputs in assignment-sorted order
    Aacc = np.empty((2 * T, D), dtype=np.float32)
    pos = np.empty(2 * T, dtype=np.int64)
    pos[ord_] = np.arange(2 * T)
    for core in range(N_CORES):
        oTe = res.results[core]["oT"]  # [128, KD, csum] f16
        for s in range(NSLOT):
            e, off, ln = assign[core][s]
            if ln == 0:
                continue
            c0 = offs[s]
            oe = oTe[:, :, c0:c0 + ln].transpose(2, 1, 0).reshape(ln, D)
            Aacc[starts[e] + off:starts[e] + off + ln] = \
                oe.astype(np.float32) + b2[e]
    out = (gates[:, :, None] *
           Aacc[pos.reshape(T, TOP_K)].astype(np.float64)).sum(axis=1)
    return out.reshape(B, S, D).astype(np.float32)
